# revision 59
# baseline (speedup 1.0000x reference)
import sys

sys.path.insert(0, "/opt/trn_rl_repo")

from contextlib import ExitStack

import numpy as np
import ml_dtypes

import concourse.bass as bass
import concourse.mybir as mybir
import concourse.tile as tile
from concourse import bacc
from concourse.bass_utils import run_bass_kernel_spmd
from concourse.masks import make_identity

H, DIM, DH = 8, 1024, 64
B, N = 2, 2048
NB = N // 128         # 16 row blocks
CC = DIM // 128       # 8 contraction chunks
CH = 256              # channels per core (2 heads x 2*DH)
LAMBDA_INIT = 0.5
RMS_EPS = 1e-5
AF = mybir.ActivationFunctionType
ALU = mybir.AluOpType
dt = mybir.dt
bf16 = ml_dtypes.bfloat16

_CACHE = {}
DEBUG = False


def _bcast_cols(ap, groups, reps):
    # [128, groups] AP -> [128, groups, reps] AP with stride-0 inner dim
    return bass.AP(tensor=ap.tensor, offset=ap.offset,
                   ap=[list(ap.ap[0]), [ap.ap[1][0], groups], [0, reps]])


def _build():
    nc = bacc.Bacc("TRN2", target_bir_lowering=False, debug=False)
    xT_d = nc.dram_tensor("xT", (DIM, N), dt.bfloat16, kind="ExternalInput").ap()
    wq_d = nc.dram_tensor("wq", (DIM, CH), dt.bfloat16, kind="ExternalInput").ap()
    wk_d = nc.dram_tensor("wk", (DIM, CH), dt.bfloat16, kind="ExternalInput").ap()
    wv_d = nc.dram_tensor("wv", (DIM, CH), dt.bfloat16, kind="ExternalInput").ap()
    wo_d = nc.dram_tensor("wo", (CH, DIM), dt.bfloat16, kind="ExternalInput").ap()
    par_d = nc.dram_tensor("par", (128, 16), dt.float32, kind="ExternalInput").ap()
    out_d = nc.dram_tensor("out", (N, DIM), dt.float32, kind="ExternalOutput").ap()
    if DEBUG:
        dqT_d = nc.dram_tensor("dqT", (128, 2, N), dt.bfloat16, kind="ExternalOutput").ap()
        dkT_d = nc.dram_tensor("dkT", (128, 2, N), dt.bfloat16, kind="ExternalOutput").ap()
        dkTD_d = nc.dram_tensor("dkTD", (128, 2, N), dt.bfloat16, kind="ExternalOutput").ap()
        dvso_d = nc.dram_tensor("dvso", (128, NB, 2, 130), dt.bfloat16, kind="ExternalOutput").ap()
        dE_d = nc.dram_tensor("dE", (128, NB, 1024), dt.bfloat16, kind="ExternalOutput").ap()
        dyT_d = nc.dram_tensor("dyT", (128, 2, N), dt.bfloat16, kind="ExternalOutput").ap()
        dyt_d = nc.dram_tensor("dyt", (128, 2, 4, 128), dt.bfloat16, kind="ExternalOutput").ap()
        dss_d = nc.dram_tensor("dss", (128, 8), dt.float32, kind="ExternalOutput").ap()
        dU_d = nc.dram_tensor("dU", (128, 2, 256), dt.float32, kind="ExternalOutput").ap()

    with tile.TileContext(nc) as tc, ExitStack() as ctx:
        persist = ctx.enter_context(tc.tile_pool(name="persist", bufs=1))
        par = persist.tile([128, 16], dt.float32)
        nc.sync.dma_start(par, par_d)
        wo_s = persist.tile([128, 2, DIM], dt.bfloat16)
        vso = persist.tile([128, NB, 2, 130], dt.bfloat16)
        nc.vector.memset(vso[:, :, :, 128:130], 1.0)
        qT = persist.tile([128, 2, N], dt.bfloat16)
        kT = persist.tile([128, 2, N], dt.bfloat16)
        kTD = persist.tile([128, 2, N], dt.bfloat16)
        yT = persist.tile([128, 2, N], dt.bfloat16)
        xT_s = persist.tile([128, CC, N], dt.bfloat16)
        wv_s = persist.tile([128, CC, CH], dt.bfloat16)

        # ---- phase 1: K and Q projections + l2-normalize (cos folded) ----
        with tc.tile_pool(name="sbpad", bufs=1) as sbpad, \
             tc.tile_pool(name="p1w", bufs=1) as p1w:
            _sbpadtile = sbpad.tile([128, 98 * 1024], dt.uint8, tag="pad")
            wk_s = p1w.tile([128, CC, CH], dt.bfloat16)
            wq_s = p1w.tile([128, CC, CH], dt.bfloat16)
            nc.sync.dma_start(wk_s, wk_d.rearrange("(c p) h -> p c h", p=128))
            nc.scalar.dma_start(wq_s, wq_d.rearrange("(c p) h -> p c h", p=128))
            for c in range(CC):
                eng = nc.sync if c % 2 == 0 else nc.scalar
                eng.dma_start(xT_s[:, c, :],
                              xT_d.rearrange("(c p) n -> p c n", p=128)[:, c, :])
            nc.sync.dma_start(wv_s, wv_d.rearrange("(c p) h -> p c h", p=128))
            nc.sync.dma_start(wo_s, wo_d.rearrange("(c p) o -> p c o", p=128))
            with tc.tile_pool(name="pspad", bufs=1, space="PSUM") as pspad, \
                 tc.tile_pool(name="ps1", bufs=4, space="PSUM") as ps1, \
                 tc.tile_pool(name="sb1", bufs=4) as sb1:
                _padtile = pspad.tile([128, 2048], dt.float32, tag="pad")

                def proj_norm(w_s, tgt, scol, tagp, kd=False):
                    for ib in range(NB):
                        bsl = slice(ib * 128, (ib + 1) * 128)
                        ps = ps1.tile([128, CH], dt.float32, tag="p")
                        for c in range(CC):
                            nc.tensor.matmul(ps, lhsT=xT_s[:, c, bsl],
                                             rhs=w_s[:, c, :],
                                             start=(c == 0), stop=(c == CC - 1),
                                             skip_group_check=True)
                        sq = sb1.tile([128, CH], dt.float32, tag=tagp + "sq")
                        nc.scalar.activation(sq, ps, AF.Square, bias=par[:, 13:14])
                        ssum = sb1.tile([128, 4], dt.float32, tag=tagp + "ss")
                        nc.vector.reduce_sum(
                            out=ssum, in_=sq.rearrange("p (g d) -> p g d", d=DH),
                            axis=mybir.AxisListType.X)
                        nrm = sb1.tile([128, 4], dt.float32, tag=tagp + "nrm")
                        nc.scalar.activation(nrm, ssum, AF.Sqrt, bias=par[:, 13:14])
                        rr = sb1.tile([128, 4], dt.float32, tag=tagp + "rr")
                        nc.vector.reciprocal(rr, nrm)
                        rs = sb1.tile([128, 4], dt.float32, tag=tagp + "rs")
                        nc.gpsimd.tensor_mul(rs, rr, par[:, scol:scol + 4])
                        nb = sb1.tile([128, CH], dt.bfloat16, tag=tagp + "nb")
                        nc.vector.tensor_tensor(
                            nb.rearrange("p (g d) -> p g d", d=DH),
                            ps.rearrange("p (g d) -> p g d", d=DH),
                            _bcast_cols(rs, 4, DH), ALU.mult)
                        for h in range(2):
                            hs = slice(h * 128, (h + 1) * 128)
                            deng = nc.sync
                            deng.dma_start_transpose(tgt[:, h, bsl], nb[:, hs])
                            if kd:
                                nc.gpsimd.tensor_scalar_mul(
                                    kTD[0:64, h, bsl], kT[0:64, h, bsl],
                                    par[0:64, 8 + 2 * h:9 + 2 * h])
                                nc.gpsimd.tensor_scalar_mul(
                                    kTD[64:128, h, bsl], kT[64:128, h, bsl],
                                    par[64:128, 9 + 2 * h:10 + 2 * h])

                proj_norm(wk_s, kT, 4, "k", kd=True)
                proj_norm(wq_s, qT, 0, "q")

        # ---- strips (n4-major) + V projection + fused out-projection ----
        with tc.tile_pool(name="Epool", bufs=2) as Epool, \
             tc.tile_pool(name="psL", bufs=2, space="PSUM") as psL, \
             tc.tile_pool(name="psD", bufs=1, space="PSUM") as psD, \
             tc.tile_pool(name="psU", bufs=2, space="PSUM") as psU, \
             tc.tile_pool(name="sbG", bufs=3) as sbG, \
             tc.tile_pool(name="sbS", bufs=3) as sbS, \
             tc.tile_pool(name="sbY", bufs=2) as sbY, \
             tc.tile_pool(name="sbO", bufs=2) as sbO:
            def v_proj(ib):
                # V projection, interleaved into strip 0 (shares psU pool)
                bsl = slice(ib * 128, (ib + 1) * 128)
                psV = psU.tile([128, 2, 256], dt.float32, tag="u")
                pv = psV.rearrange("p a b -> p (a b)")[:, 0:256]
                for c in range(CC):
                    nc.tensor.matmul(pv, lhsT=xT_s[:, c, bsl], rhs=wv_s[:, c, :],
                                     start=(c == 0), stop=(c == CC - 1),
                                     skip_group_check=True)
                nc.vector.tensor_copy(
                    vso[:, ib, :, 0:128],
                    pv.rearrange("p (h d) -> p h d", d=128))

            def finalize(n4, ytmp, ssqa):
                # batched rho for this n4 (both heads, 8 query blocks)
                rho = sbY.tile([128, 8], dt.float32, tag="rho")
                nc.scalar.activation(rho, ssqa, AF.Sqrt, scale=1.0 / 128.0,
                                     bias=par[:, 14:15])
                rrho = sbY.tile([128, 8], dt.float32, tag="rrho")
                nc.vector.reciprocal(rrho, rho)
                for h in range(2):
                    for qc in range(4):
                        yq = sbY.tile([128, 128], dt.bfloat16, tag="yq")
                        nc.gpsimd.tensor_scalar_mul(
                            yq, ytmp[:, h, qc, :], rrho[:, 4 * h + qc:4 * h + qc + 1])
                        nc.sync.dma_start_transpose(
                            yT[:, h, n4 * 512 + qc * 128:n4 * 512 + (qc + 1) * 128],
                            yq)
                # out projection for this n4's four row blocks (psU-shared)
                for ib in range(4 * n4, 4 * n4 + 4):
                    bsl = slice(ib * 128, (ib + 1) * 128)
                    for half in range(2):
                        osl = slice(half * 512, (half + 1) * 512)
                        pOt = psU.tile([128, 2, 256], dt.float32, tag="u")
                        pO = pOt.rearrange("p a b -> p (a b)")
                        for h in range(2):
                            nc.tensor.matmul(pO, lhsT=yT[:, h, bsl],
                                             rhs=wo_s[:, h, osl],
                                             start=(h == 0), stop=(h == 1),
                                             skip_group_check=True)
                        ob = sbO.tile([128, 512], dt.float32, tag="ob")
                        if half == 0:
                            nc.vector.tensor_copy(ob, pO)
                        else:
                            nc.scalar.activation(ob, pO, AF.Copy)
                        nc.sync.dma_start(out_d[bsl, osl], ob)

            for n4 in range(4):
                nsl = slice(n4 * 512, (n4 + 1) * 512)
                ytmp = sbY.tile([128, 2, 4, 128], dt.bfloat16, tag="ytmp")
                ssqa = sbY.tile([128, 8], dt.float32, tag="ssqa")
                for h in range(2):
                    E = Epool.tile([128, NB, 1024], dt.bfloat16, tag="E")
                    for jp in range(NB // 2):
                        Dp = psD.tile([128, 1024], dt.float32, tag="D")
                        Gp = sbG.tile([128, 1024], dt.float32, tag="G")
                        S12p = sbS.tile([128, 2, 1024], dt.bfloat16, tag="S")
                        for j2 in range(2):
                            im = 2 * jp + j2
                            msl = slice(im * 128, (im + 1) * 128)
                            nc.tensor.matmul(Dp[:, j2 * 512:(j2 + 1) * 512],
                                             lhsT=kTD[:, h, msl], rhs=qT[:, h, nsl],
                                             start=True, stop=True,
                                             skip_group_check=True)
                        nc.scalar.activation(Gp, Dp, AF.Tanh, bias=par[:, 13:14])
                        for j2 in range(2):
                            im = 2 * jp + j2
                            msl = slice(im * 128, (im + 1) * 128)
                            gsl = slice(j2 * 512, (j2 + 1) * 512)
                            L12 = psL.tile([128, 1024], dt.float32, tag="L")
                            nc.tensor.matmul(L12[:, 0:512], lhsT=kT[0:64, h, msl],
                                             rhs=qT[0:64, h, nsl],
                                             start=True, stop=True,
                                             skip_group_check=True)
                            nc.tensor.matmul(L12[:, 512:1024],
                                             lhsT=kT[64:128, h, msl],
                                             rhs=qT[64:128, h, nsl],
                                             start=True, stop=True,
                                             skip_group_check=True)
                            nc.vector.tensor_add(S12p[:, j2, 0:512],
                                                 L12[:, 0:512], Gp[:, gsl])
                            nc.vector.scalar_tensor_tensor(
                                S12p[:, j2, 512:1024], Gp[:, gsl], -1.0,
                                L12[:, 512:1024], ALU.mult, ALU.add)
                        nc.scalar.activation(E[:, 2 * jp:2 * jp + 2, :], S12p,
                                             AF.Exp, bias=par[:, 13:14])
                        if n4 == 0 and h == 0:
                            v_proj(2 * jp)
                            v_proj(2 * jp + 1)
                    if DEBUG and n4 == 0 and h == 0:
                        nc.sync.dma_start(dE_d, E)
                    for qc in range(4):
                        Up = psU.tile([128, 2, 256], dt.float32, tag="u")
                        q0 = qc * 128
                        # sequential accumulation groups: the whole 2KB PSUM
                        # bank is one zero-region, so the two groups must not
                        # interleave their start/stop windows
                        for br in range(2):
                            for im in range(NB):
                                st, sp = (im == 0), (im == NB - 1)
                                nc.tensor.matmul(Up[:, br, 0:129],
                                                 lhsT=E[:, im, 512 * br + q0:
                                                         512 * br + q0 + 128],
                                                 rhs=vso[:, im, h, 0:129],
                                                 start=st, stop=sp,
                                                 skip_group_check=True)
                        if DEBUG and n4 == 0 and h == 0 and qc == 0:
                            dUc = sbY.tile([128, 2, 256], dt.float32, tag="dbgU")
                            nc.vector.memset(dUc, 0.0)
                            nc.vector.tensor_copy(dUc[:, :, 0:129], Up[:, :, 0:129])
                            nc.sync.dma_start(dU_d, dUc)
                        rab = sbY.tile([128, 2], dt.float32, tag="rab")
                        nc.vector.reciprocal(rab, Up[:, :, 128:129])
                        rb = sbY.tile([128, 1], dt.float32, tag="rb")
                        nc.vector.tensor_scalar_mul(rb, rab[:, 1:2], par[:, 12:13])
                        t2 = sbY.tile([128, 128], dt.bfloat16, tag="t2")
                        nc.vector.tensor_scalar_mul(t2, Up[:, 1, 0:128], rb)
                        y = ytmp[:, h, qc, :]
                        nc.vector.scalar_tensor_tensor(y, Up[:, 0, 0:128],
                                                       rab[:, 0:1], t2,
                                                       ALU.mult, ALU.subtract)
                        scr = sbY.tile([128, 128], dt.bfloat16, tag="scr")
                        nc.gpsimd.tensor_mul(scr, y, y)
                        nc.vector.reduce_sum(
                            out=ssqa[:, 4 * h + qc:4 * h + qc + 1],
                            in_=scr, axis=mybir.AxisListType.X)
                if DEBUG and n4 == 0:
                    nc.sync.dma_start(dyt_d, ytmp)
                    nc.sync.dma_start(dss_d, ssqa)
                finalize(n4, ytmp, ssqa)
            if DEBUG:
                for t_s, t_d in ((qT, dqT_d), (kT, dkT_d), (kTD, dkTD_d), (yT, dyT_d)):
                    nc.sync.dma_start(t_d, t_s)
                nc.sync.dma_start(dvso_d, vso)

    nc.compile()
    return nc


def kernel(x, Wq, Wk, Wv, Wo, bo,
           lambda_q1, lambda_k1, lambda_q2, lambda_k2,
           delta_gain, cos_head_delta, cos_logit_scale_raw, subln_weight,
           trace=False):
    x = np.asarray(x, np.float32)
    Wq = np.asarray(Wq, np.float32)
    Wk = np.asarray(Wk, np.float32)
    Wv = np.asarray(Wv, np.float32)
    Wo = np.asarray(Wo, np.float32)
    bo = np.asarray(bo, np.float32)

    raw = np.float32(cos_logit_scale_raw)
    gscale = 15.0 / (1.0 + np.exp(-raw))
    hd = np.asarray(cos_head_delta, np.float32)
    hd = hd - hd.mean()
    cos_scale = (gscale * (1.0 + 0.5 * np.tanh(hd))).astype(np.float32)  # (H,)
    lam = np.float32(
        np.exp(np.sum(np.asarray(lambda_q1, np.float32) * np.asarray(lambda_k1, np.float32)))
        - np.exp(np.sum(np.asarray(lambda_q2, np.float32) * np.asarray(lambda_k2, np.float32)))
        + LAMBDA_INIT)
    dg = np.asarray(delta_gain, np.float32)
    wsub = (np.asarray(subln_weight, np.float32) * (1.0 - LAMBDA_INIT)).astype(np.float32)
    wsub_ch = np.tile(wsub, 2)  # (256,) per-core channel scale

    if "nc" not in _CACHE:
        _CACHE["nc"] = _build()
    nc = _CACHE["nc"]

    xTb = [np.ascontiguousarray(x[b].T).astype(bf16) for b in range(B)]
    in_maps = []
    for core in range(8):
        b, g = core // 4, core % 4
        h0 = 2 * g
        rows = slice(h0 * 2 * DH, (h0 + 2) * 2 * DH)  # 256 channels
        par = np.zeros((128, 16), np.float32)
        par[:, 0] = cos_scale[h0]
        par[:, 1] = cos_scale[h0]
        par[:, 2] = cos_scale[h0 + 1]
        par[:, 3] = cos_scale[h0 + 1]
        par[:, 4:8] = 1.0
        par[:, 8] = dg[h0]
        par[:, 9] = -dg[h0]
        par[:, 10] = dg[h0 + 1]
        par[:, 11] = -dg[h0 + 1]
        par[:, 12] = lam
        par[:, 14] = RMS_EPS
        wo_c = np.ascontiguousarray(Wo[:, rows].T) * wsub_ch[:, None]
        in_maps.append({
            "xT": xTb[b],
            "wq": np.ascontiguousarray(Wq[rows].T).astype(bf16),
            "wk": np.ascontiguousarray(Wk[rows].T).astype(bf16),
            "wv": np.ascontiguousarray(Wv[rows].T).astype(bf16),
            "wo": wo_c.astype(bf16),
            "par": par,
        })

    res = run_bass_kernel_spmd(nc, in_maps, core_ids=list(range(8)), trace=trace)
    outs = [res.results[c]["out"] for c in range(8)]
    full = np.zeros((B, N, DIM), np.float32)
    for b in range(B):
        acc = outs[4 * b].astype(np.float32)
        for g in range(1, 4):
            acc = acc + outs[4 * b + g].astype(np.float32)
        full[b] = acc + bo[None, :]
    if trace:
        return full, res
    return full


# revision 60
# speedup vs baseline: 1.0031x; 1.0031x over previous
import sys

sys.path.insert(0, "/opt/trn_rl_repo")

from contextlib import ExitStack

import numpy as np
import ml_dtypes

import concourse.bass as bass
import concourse.mybir as mybir
import concourse.tile as tile
from concourse import bacc
from concourse.bass_utils import run_bass_kernel_spmd
from concourse.masks import make_identity

H, DIM, DH = 8, 1024, 64
B, N = 2, 2048
NB = N // 128         # 16 row blocks
CC = DIM // 128       # 8 contraction chunks
CH = 256              # channels per core (2 heads x 2*DH)
LAMBDA_INIT = 0.5
RMS_EPS = 1e-5
AF = mybir.ActivationFunctionType
ALU = mybir.AluOpType
dt = mybir.dt
bf16 = ml_dtypes.bfloat16

_CACHE = {}
DEBUG = False


def _bcast_cols(ap, groups, reps):
    # [128, groups] AP -> [128, groups, reps] AP with stride-0 inner dim
    return bass.AP(tensor=ap.tensor, offset=ap.offset,
                   ap=[list(ap.ap[0]), [ap.ap[1][0], groups], [0, reps]])


def _build():
    nc = bacc.Bacc("TRN2", target_bir_lowering=False, debug=False)
    xT_d = nc.dram_tensor("xT", (DIM, N), dt.bfloat16, kind="ExternalInput").ap()
    wq_d = nc.dram_tensor("wq", (DIM, CH), dt.bfloat16, kind="ExternalInput").ap()
    wk_d = nc.dram_tensor("wk", (DIM, CH), dt.bfloat16, kind="ExternalInput").ap()
    wv_d = nc.dram_tensor("wv", (DIM, CH), dt.bfloat16, kind="ExternalInput").ap()
    wo_d = nc.dram_tensor("wo", (CH, DIM), dt.bfloat16, kind="ExternalInput").ap()
    par_d = nc.dram_tensor("par", (128, 16), dt.float32, kind="ExternalInput").ap()
    out_d = nc.dram_tensor("out", (N, DIM), dt.float32, kind="ExternalOutput").ap()
    if DEBUG:
        dqT_d = nc.dram_tensor("dqT", (128, 2, N), dt.bfloat16, kind="ExternalOutput").ap()
        dkT_d = nc.dram_tensor("dkT", (128, 2, N), dt.bfloat16, kind="ExternalOutput").ap()
        dkTD_d = nc.dram_tensor("dkTD", (128, 2, N), dt.bfloat16, kind="ExternalOutput").ap()
        dvso_d = nc.dram_tensor("dvso", (128, NB, 2, 130), dt.bfloat16, kind="ExternalOutput").ap()
        dE_d = nc.dram_tensor("dE", (128, NB, 1024), dt.bfloat16, kind="ExternalOutput").ap()
        dyT_d = nc.dram_tensor("dyT", (128, 2, N), dt.bfloat16, kind="ExternalOutput").ap()
        dyt_d = nc.dram_tensor("dyt", (128, 2, 4, 128), dt.bfloat16, kind="ExternalOutput").ap()
        dss_d = nc.dram_tensor("dss", (128, 8), dt.float32, kind="ExternalOutput").ap()
        dU_d = nc.dram_tensor("dU", (128, 2, 256), dt.float32, kind="ExternalOutput").ap()

    with tile.TileContext(nc) as tc, ExitStack() as ctx:
        persist = ctx.enter_context(tc.tile_pool(name="persist", bufs=1))
        par = persist.tile([128, 16], dt.float32)
        nc.sync.dma_start(par, par_d)
        wo_s = persist.tile([128, 2, DIM], dt.bfloat16)
        vso = persist.tile([128, NB, 2, 130], dt.bfloat16)
        nc.vector.memset(vso[:, :, :, 128:130], 1.0)
        qT = persist.tile([128, 2, N], dt.bfloat16)
        kT = persist.tile([128, 2, N], dt.bfloat16)
        kTD = persist.tile([128, 2, N], dt.bfloat16)
        yT = persist.tile([128, 2, N], dt.bfloat16)
        xT_s = persist.tile([128, CC, N], dt.bfloat16)
        wv_s = persist.tile([128, CC, CH], dt.bfloat16)

        # ---- phase 1: K and Q projections + l2-normalize (cos folded) ----
        with tc.tile_pool(name="sbpad", bufs=1) as sbpad, \
             tc.tile_pool(name="p1w", bufs=1) as p1w:
            _sbpadtile = sbpad.tile([128, 98 * 1024], dt.uint8, tag="pad")
            wk_s = p1w.tile([128, CC, CH], dt.bfloat16)
            wq_s = p1w.tile([128, CC, CH], dt.bfloat16)
            nc.sync.dma_start(wk_s, wk_d.rearrange("(c p) h -> p c h", p=128))
            nc.scalar.dma_start(wq_s, wq_d.rearrange("(c p) h -> p c h", p=128))
            for c in range(CC):
                eng = nc.sync if c % 2 == 0 else nc.scalar
                eng.dma_start(xT_s[:, c, :],
                              xT_d.rearrange("(c p) n -> p c n", p=128)[:, c, :])
            nc.sync.dma_start(wv_s, wv_d.rearrange("(c p) h -> p c h", p=128))
            nc.sync.dma_start(wo_s, wo_d.rearrange("(c p) o -> p c o", p=128))
            with tc.tile_pool(name="pspad", bufs=1, space="PSUM") as pspad, \
                 tc.tile_pool(name="ps1", bufs=4, space="PSUM") as ps1, \
                 tc.tile_pool(name="sb1", bufs=4) as sb1:
                _padtile = pspad.tile([128, 2048], dt.float32, tag="pad")

                def proj_norm(w_s, tgt, scol, tagp, kd=False):
                    for ib in range(NB):
                        bsl = slice(ib * 128, (ib + 1) * 128)
                        ps = ps1.tile([128, CH], dt.float32, tag="p")
                        for c in range(CC):
                            nc.tensor.matmul(ps, lhsT=xT_s[:, c, bsl],
                                             rhs=w_s[:, c, :],
                                             start=(c == 0), stop=(c == CC - 1),
                                             skip_group_check=True)
                        sq = sb1.tile([128, CH], dt.float32, tag=tagp + "sq")
                        nc.scalar.activation(sq, ps, AF.Square, bias=par[:, 13:14])
                        ssum = sb1.tile([128, 4], dt.float32, tag=tagp + "ss")
                        nc.vector.reduce_sum(
                            out=ssum, in_=sq.rearrange("p (g d) -> p g d", d=DH),
                            axis=mybir.AxisListType.X)
                        nrm = sb1.tile([128, 4], dt.float32, tag=tagp + "nrm")
                        nc.scalar.activation(nrm, ssum, AF.Sqrt, bias=par[:, 13:14])
                        rr = sb1.tile([128, 4], dt.float32, tag=tagp + "rr")
                        nc.vector.reciprocal(rr, nrm)
                        rs = sb1.tile([128, 4], dt.float32, tag=tagp + "rs")
                        nc.gpsimd.tensor_mul(rs, rr, par[:, scol:scol + 4])
                        nb = sb1.tile([128, CH], dt.bfloat16, tag=tagp + "nb")
                        nc.vector.tensor_tensor(
                            nb.rearrange("p (g d) -> p g d", d=DH),
                            ps.rearrange("p (g d) -> p g d", d=DH),
                            _bcast_cols(rs, 4, DH), ALU.mult)
                        for h in range(2):
                            hs = slice(h * 128, (h + 1) * 128)
                            deng = nc.sync
                            deng.dma_start_transpose(tgt[:, h, bsl], nb[:, hs])
                            if kd:
                                nc.gpsimd.tensor_scalar_mul(
                                    kTD[0:64, h, bsl], kT[0:64, h, bsl],
                                    par[0:64, 8 + 2 * h:9 + 2 * h])
                                nc.gpsimd.tensor_scalar_mul(
                                    kTD[64:128, h, bsl], kT[64:128, h, bsl],
                                    par[64:128, 9 + 2 * h:10 + 2 * h])

                proj_norm(wq_s, qT, 0, "q")
                proj_norm(wk_s, kT, 4, "k", kd=True)

        # ---- strips (n4-major) + V projection + fused out-projection ----
        with tc.tile_pool(name="Epool", bufs=2) as Epool, \
             tc.tile_pool(name="psL", bufs=2, space="PSUM") as psL, \
             tc.tile_pool(name="psD", bufs=1, space="PSUM") as psD, \
             tc.tile_pool(name="psU", bufs=2, space="PSUM") as psU, \
             tc.tile_pool(name="sbG", bufs=3) as sbG, \
             tc.tile_pool(name="sbS", bufs=3) as sbS, \
             tc.tile_pool(name="sbY", bufs=2) as sbY, \
             tc.tile_pool(name="sbO", bufs=2) as sbO:
            def v_proj(ib):
                # V projection, interleaved into strip 0 (shares psU pool)
                bsl = slice(ib * 128, (ib + 1) * 128)
                psV = psU.tile([128, 2, 256], dt.float32, tag="u")
                pv = psV.rearrange("p a b -> p (a b)")[:, 0:256]
                for c in range(CC):
                    nc.tensor.matmul(pv, lhsT=xT_s[:, c, bsl], rhs=wv_s[:, c, :],
                                     start=(c == 0), stop=(c == CC - 1),
                                     skip_group_check=True)
                nc.vector.tensor_copy(
                    vso[:, ib, :, 0:128],
                    pv.rearrange("p (h d) -> p h d", d=128))

            def finalize(n4, ytmp, ssqa):
                # batched rho for this n4 (both heads, 8 query blocks)
                rho = sbY.tile([128, 8], dt.float32, tag="rho")
                nc.scalar.activation(rho, ssqa, AF.Sqrt, scale=1.0 / 128.0,
                                     bias=par[:, 14:15])
                rrho = sbY.tile([128, 8], dt.float32, tag="rrho")
                nc.vector.reciprocal(rrho, rho)
                for h in range(2):
                    for qc in range(4):
                        yq = sbY.tile([128, 128], dt.bfloat16, tag="yq")
                        nc.gpsimd.tensor_scalar_mul(
                            yq, ytmp[:, h, qc, :], rrho[:, 4 * h + qc:4 * h + qc + 1])
                        nc.sync.dma_start_transpose(
                            yT[:, h, n4 * 512 + qc * 128:n4 * 512 + (qc + 1) * 128],
                            yq)
                # out projection for this n4's four row blocks (psU-shared)
                for ib in range(4 * n4, 4 * n4 + 4):
                    bsl = slice(ib * 128, (ib + 1) * 128)
                    for half in range(2):
                        osl = slice(half * 512, (half + 1) * 512)
                        pOt = psU.tile([128, 2, 256], dt.float32, tag="u")
                        pO = pOt.rearrange("p a b -> p (a b)")
                        for h in range(2):
                            nc.tensor.matmul(pO, lhsT=yT[:, h, bsl],
                                             rhs=wo_s[:, h, osl],
                                             start=(h == 0), stop=(h == 1),
                                             skip_group_check=True)
                        ob = sbO.tile([128, 512], dt.float32, tag="ob")
                        if half == 0:
                            nc.vector.tensor_copy(ob, pO)
                        else:
                            nc.scalar.activation(ob, pO, AF.Copy)
                        nc.sync.dma_start(out_d[bsl, osl], ob)

            for n4 in range(4):
                nsl = slice(n4 * 512, (n4 + 1) * 512)
                ytmp = sbY.tile([128, 2, 4, 128], dt.bfloat16, tag="ytmp")
                ssqa = sbY.tile([128, 8], dt.float32, tag="ssqa")
                for h in range(2):
                    E = Epool.tile([128, NB, 1024], dt.bfloat16, tag="E")
                    for jp in range(NB // 2):
                        Dp = psD.tile([128, 1024], dt.float32, tag="D")
                        Gp = sbG.tile([128, 1024], dt.float32, tag="G")
                        S12p = sbS.tile([128, 2, 1024], dt.bfloat16, tag="S")
                        for j2 in range(2):
                            im = 2 * jp + j2
                            msl = slice(im * 128, (im + 1) * 128)
                            nc.tensor.matmul(Dp[:, j2 * 512:(j2 + 1) * 512],
                                             lhsT=kTD[:, h, msl], rhs=qT[:, h, nsl],
                                             start=True, stop=True,
                                             skip_group_check=True)
                        nc.scalar.activation(Gp, Dp, AF.Tanh, bias=par[:, 13:14])
                        for j2 in range(2):
                            im = 2 * jp + j2
                            msl = slice(im * 128, (im + 1) * 128)
                            gsl = slice(j2 * 512, (j2 + 1) * 512)
                            L12 = psL.tile([128, 1024], dt.float32, tag="L")
                            nc.tensor.matmul(L12[:, 0:512], lhsT=kT[0:64, h, msl],
                                             rhs=qT[0:64, h, nsl],
                                             start=True, stop=True,
                                             skip_group_check=True)
                            nc.tensor.matmul(L12[:, 512:1024],
                                             lhsT=kT[64:128, h, msl],
                                             rhs=qT[64:128, h, nsl],
                                             start=True, stop=True,
                                             skip_group_check=True)
                            nc.vector.tensor_add(S12p[:, j2, 0:512],
                                                 L12[:, 0:512], Gp[:, gsl])
                            nc.vector.scalar_tensor_tensor(
                                S12p[:, j2, 512:1024], Gp[:, gsl], -1.0,
                                L12[:, 512:1024], ALU.mult, ALU.add)
                        nc.scalar.activation(E[:, 2 * jp:2 * jp + 2, :], S12p,
                                             AF.Exp, bias=par[:, 13:14])
                        if n4 == 0 and h == 0:
                            v_proj(2 * jp)
                            v_proj(2 * jp + 1)
                    if DEBUG and n4 == 0 and h == 0:
                        nc.sync.dma_start(dE_d, E)
                    for qc in range(4):
                        Up = psU.tile([128, 2, 256], dt.float32, tag="u")
                        q0 = qc * 128
                        # sequential accumulation groups: the whole 2KB PSUM
                        # bank is one zero-region, so the two groups must not
                        # interleave their start/stop windows
                        for br in range(2):
                            for im in range(NB):
                                st, sp = (im == 0), (im == NB - 1)
                                nc.tensor.matmul(Up[:, br, 0:129],
                                                 lhsT=E[:, im, 512 * br + q0:
                                                         512 * br + q0 + 128],
                                                 rhs=vso[:, im, h, 0:129],
                                                 start=st, stop=sp,
                                                 skip_group_check=True)
                        if DEBUG and n4 == 0 and h == 0 and qc == 0:
                            dUc = sbY.tile([128, 2, 256], dt.float32, tag="dbgU")
                            nc.vector.memset(dUc, 0.0)
                            nc.vector.tensor_copy(dUc[:, :, 0:129], Up[:, :, 0:129])
                            nc.sync.dma_start(dU_d, dUc)
                        rab = sbY.tile([128, 2], dt.float32, tag="rab")
                        nc.vector.reciprocal(rab, Up[:, :, 128:129])
                        rb = sbY.tile([128, 1], dt.float32, tag="rb")
                        nc.vector.tensor_scalar_mul(rb, rab[:, 1:2], par[:, 12:13])
                        t2 = sbY.tile([128, 128], dt.bfloat16, tag="t2")
                        nc.vector.tensor_scalar_mul(t2, Up[:, 1, 0:128], rb)
                        y = ytmp[:, h, qc, :]
                        nc.vector.scalar_tensor_tensor(y, Up[:, 0, 0:128],
                                                       rab[:, 0:1], t2,
                                                       ALU.mult, ALU.subtract)
                        scr = sbY.tile([128, 128], dt.bfloat16, tag="scr")
                        nc.gpsimd.tensor_mul(scr, y, y)
                        nc.vector.reduce_sum(
                            out=ssqa[:, 4 * h + qc:4 * h + qc + 1],
                            in_=scr, axis=mybir.AxisListType.X)
                if DEBUG and n4 == 0:
                    nc.sync.dma_start(dyt_d, ytmp)
                    nc.sync.dma_start(dss_d, ssqa)
                finalize(n4, ytmp, ssqa)
            if DEBUG:
                for t_s, t_d in ((qT, dqT_d), (kT, dkT_d), (kTD, dkTD_d), (yT, dyT_d)):
                    nc.sync.dma_start(t_d, t_s)
                nc.sync.dma_start(dvso_d, vso)

    nc.compile()
    return nc


def kernel(x, Wq, Wk, Wv, Wo, bo,
           lambda_q1, lambda_k1, lambda_q2, lambda_k2,
           delta_gain, cos_head_delta, cos_logit_scale_raw, subln_weight,
           trace=False):
    x = np.asarray(x, np.float32)
    Wq = np.asarray(Wq, np.float32)
    Wk = np.asarray(Wk, np.float32)
    Wv = np.asarray(Wv, np.float32)
    Wo = np.asarray(Wo, np.float32)
    bo = np.asarray(bo, np.float32)

    raw = np.float32(cos_logit_scale_raw)
    gscale = 15.0 / (1.0 + np.exp(-raw))
    hd = np.asarray(cos_head_delta, np.float32)
    hd = hd - hd.mean()
    cos_scale = (gscale * (1.0 + 0.5 * np.tanh(hd))).astype(np.float32)  # (H,)
    lam = np.float32(
        np.exp(np.sum(np.asarray(lambda_q1, np.float32) * np.asarray(lambda_k1, np.float32)))
        - np.exp(np.sum(np.asarray(lambda_q2, np.float32) * np.asarray(lambda_k2, np.float32)))
        + LAMBDA_INIT)
    dg = np.asarray(delta_gain, np.float32)
    wsub = (np.asarray(subln_weight, np.float32) * (1.0 - LAMBDA_INIT)).astype(np.float32)
    wsub_ch = np.tile(wsub, 2)  # (256,) per-core channel scale

    if "nc" not in _CACHE:
        _CACHE["nc"] = _build()
    nc = _CACHE["nc"]

    xTb = [np.ascontiguousarray(x[b].T).astype(bf16) for b in range(B)]
    in_maps = []
    for core in range(8):
        b, g = core // 4, core % 4
        h0 = 2 * g
        rows = slice(h0 * 2 * DH, (h0 + 2) * 2 * DH)  # 256 channels
        par = np.zeros((128, 16), np.float32)
        par[:, 0] = cos_scale[h0]
        par[:, 1] = cos_scale[h0]
        par[:, 2] = cos_scale[h0 + 1]
        par[:, 3] = cos_scale[h0 + 1]
        par[:, 4:8] = 1.0
        par[:, 8] = dg[h0]
        par[:, 9] = -dg[h0]
        par[:, 10] = dg[h0 + 1]
        par[:, 11] = -dg[h0 + 1]
        par[:, 12] = lam
        par[:, 14] = RMS_EPS
        wo_c = np.ascontiguousarray(Wo[:, rows].T) * wsub_ch[:, None]
        in_maps.append({
            "xT": xTb[b],
            "wq": np.ascontiguousarray(Wq[rows].T).astype(bf16),
            "wk": np.ascontiguousarray(Wk[rows].T).astype(bf16),
            "wv": np.ascontiguousarray(Wv[rows].T).astype(bf16),
            "wo": wo_c.astype(bf16),
            "par": par,
        })

    res = run_bass_kernel_spmd(nc, in_maps, core_ids=list(range(8)), trace=trace)
    outs = [res.results[c]["out"] for c in range(8)]
    full = np.zeros((B, N, DIM), np.float32)
    for b in range(B):
        acc = outs[4 * b].astype(np.float32)
        for g in range(1, 4):
            acc = acc + outs[4 * b + g].astype(np.float32)
        full[b] = acc + bo[None, :]
    if trace:
        return full, res
    return full
